# revision 1
# baseline (speedup 1.0000x reference)
"""Contrastive-loss kernel for 8 TRN2 NeuronCores (Bass/Tile).

loss = sum_{i!=j}[ same(i,j)*d2(i,j) + diff(i,j)*relu(1-d(i,j))^2 ] / (n(n-1))

Decomposition:
  P = sum over same-label pairs of d2  ==  sum_c (2*n_c*S_c - 2*|M_c|^2)
      (exact identity; per-class count / sum |x|^2 / sum x computed on-device
       with fp32 matmuls over each core's row strip)
  Q = sum over diff-label pairs of relu(1-d)^2 -- nonzero only if some
      diff-label pair has d2 < 1.  The device certifies Q == 0 by scanning
      every unordered pair once: s(i,j) = s(j,i), so row-tile g (128 rows)
      only scans the cyclic column band [128g, 128g+4096) (col-block
      distances 0..31) plus the distance-32 block [128g+4096, +128).
      Each PSUM tile holds t = -(d2 + 256*m) from one bf16 augmented matmul
      (m = same-label mask via 16*onehot rows folded into the contraction).
      ScalarE consumes band half A with Relu(t+THETA)+accum_out, VectorE
      reduce_max's half B.  Any flag => exact host recompute of Q.

Per-core rhs is ROTATED by 1024*c cols so every core's band starts at
local col 0 (uniform DMA start, 5120-col rhs instead of 8192).
"""

import numpy as np
import ml_dtypes

import concourse.bass as bass
import concourse.bacc as bacc
import concourse.tile as tile
from concourse import mybir
from concourse.bass_utils import run_bass_kernel_spmd

MARGIN = 1.0

N, D, NCLS, CORES = 8192, 64, 8, 8
ROWS = N // CORES            # rows per core
PT = 128                     # rows per row-tile
RT = ROWS // PT              # row-tiles per core
BAND = 4096                  # cyclic band columns per row-tile (dist 0..31)
EXT = 128                    # distance-32 extras block
RLOC = BAND + RT * PT        # local rhs width: 4096 + 1024 ... see below
RHSW = (RT - 1) * PT + BAND + EXT   # 896 + 4096 + 128 = 5120
MMN = 512                    # matmul free dim (one PSUM bank)
PSW = 1024                   # PSUM tile width (2 banks; bufs=4 => full PSUM)
NTPR = BAND // PSW           # psum tiles per row-tile (4)
KAUG = D + 2 + NCLS          # 74
MSCALE = 16.0                # onehot scale; same-label mask adds 256
THETA = 2.0                  # flag threshold on d2
NCHUNK = ROWS // PT          # class-sum K chunks
FDIM = D + 2                 # [x | sq | 1]
NVIOL = RT * NTPR + 1        # viol columns


def build_nc(repeats: int = 1):
    nc = bacc.Bacc("TRN2", target_bir_lowering=False, debug=False,
                   num_devices=CORES)
    bf16, f32 = mybir.dt.bfloat16, mybir.dt.float32

    lhst_d = nc.dram_tensor("lhst", [KAUG, ROWS], bf16, kind="ExternalInput")
    rhs_d = nc.dram_tensor("rhs", [KAUG, RHSW], bf16, kind="ExternalInput")
    clsoh_d = nc.dram_tensor("clsoh", [PT, NCHUNK, NCLS], f32,
                             kind="ExternalInput")
    clsft_d = nc.dram_tensor("clsft", [PT, NCHUNK, FDIM], f32,
                             kind="ExternalInput")
    viol_d = nc.dram_tensor("viol", [PT, NVIOL], f32, kind="ExternalOutput")
    cls_d = nc.dram_tensor("cls", [NCLS, FDIM], f32, kind="ExternalOutput")

    with tile.TileContext(nc) as tc:
        with (
            tc.tile_pool(name="w", bufs=1) as wpool,
            tc.tile_pool(name="ps", bufs=4, space="PSUM") as pspool,
            tc.tile_pool(name="scr", bufs=2) as scrpool,
            tc.tile_pool(name="acc", bufs=1) as accpool,
        ):
            lhst = wpool.tile([KAUG, ROWS], bf16)
            nc.sync.dma_start(out=lhst[:], in_=lhst_d[:])
            rhs = wpool.tile([KAUG, RHSW], bf16)
            for a, b in ((0, 1024), (1024, 2048), (2048, 3072),
                         (3072, 4096), (4096, RHSW)):
                nc.sync.dma_start(out=rhs[:, a:b], in_=rhs_d[:, a:b])
            clsoh = wpool.tile([PT, NCHUNK, NCLS], f32)
            nc.sync.dma_start(out=clsoh[:], in_=clsoh_d[:])
            clsft = wpool.tile([PT, NCHUNK, FDIM], f32)
            nc.sync.dma_start(out=clsft[:], in_=clsft_d[:])

            viol_sb = accpool.tile([PT, NVIOL], f32)
            cls_sb = accpool.tile([NCLS, FDIM], f32)
            theta_sb = accpool.tile([PT, 1], f32)
            nc.vector.memset(theta_sb, THETA)

            for _rep in range(repeats):
                for r in range(RT):
                    base = r * PT
                    for j in range(NTPR):
                        ps = pspool.tile([PT, PSW], f32, tag="ps")
                        for k in range(PSW // MMN):
                            off = base + j * PSW + k * MMN
                            nc.tensor.matmul(
                                ps[:, k * MMN:(k + 1) * MMN],
                                lhst[:, base:base + PT],
                                rhs[:, off:off + MMN],
                                start=True, stop=True)
                        c = r * NTPR + j
                        if j % 2 == 0:   # ScalarE: sum of relu(t+theta)
                            scr = scrpool.tile([PT, PSW], f32, tag="scr")
                            nc.scalar.activation(
                                out=scr[:], in_=ps[:],
                                func=mybir.ActivationFunctionType.Relu,
                                bias=theta_sb[:], scale=1.0,
                                accum_out=viol_sb[:, c:c + 1])
                        else:            # VectorE: max of t
                            nc.vector.tensor_reduce(
                                out=viol_sb[:, c:c + 1], in_=ps[:],
                                axis=mybir.AxisListType.X,
                                op=mybir.AluOpType.max)

                # distance-32 extras: one 128-col block per row-tile -> DVE
                pse = pspool.tile([PT, PSW], f32, tag="ps")
                for r in range(RT):
                    nc.tensor.matmul(
                        pse[:, r * EXT:(r + 1) * EXT],
                        lhst[:, r * PT:(r + 1) * PT],
                        rhs[:, r * PT + BAND:r * PT + BAND + EXT],
                        start=True, stop=True)
                nc.vector.tensor_reduce(
                    out=viol_sb[:, RT * NTPR:RT * NTPR + 1], in_=pse[:],
                    axis=mybir.AxisListType.X, op=mybir.AluOpType.max)

                # class sums (fp32)
                psc = pspool.tile([PT, PSW], f32, tag="ps")
                for i in range(NCHUNK):
                    nc.tensor.matmul(
                        psc[:NCLS, 0:FDIM],
                        clsoh[:, i, :],
                        clsft[:, i, :],
                        start=(i == 0), stop=(i == NCHUNK - 1))
                nc.scalar.copy(out=cls_sb[:], in_=psc[:NCLS, 0:FDIM])

            nc.sync.dma_start(out=viol_d[:], in_=viol_sb[:])
            nc.sync.dma_start(out=cls_d[:], in_=cls_sb[:])
    nc.compile()
    return nc


def prep_inputs(x: np.ndarray, label: np.ndarray):
    """Host-side sharding prep: augmented bf16 matrices (rhs rotated per
    core) + fp32 class-sum operands."""
    x64 = x.astype(np.float64)
    sq = (x64 * x64).sum(axis=1)
    oh = np.zeros((N, NCLS), np.float64)
    oh[np.arange(N), label] = 1.0

    lhst_all = np.concatenate(
        [x64, sq[:, None], np.ones((N, 1)), MSCALE * oh], axis=1
    ).T.astype(ml_dtypes.bfloat16)                     # [KAUG, N]
    rhs_all = np.concatenate(
        [2.0 * x64, -np.ones((N, 1)), -sq[:, None], -MSCALE * oh], axis=1
    ).T.astype(ml_dtypes.bfloat16)                     # [KAUG, N]
    rhs2 = np.concatenate([rhs_all, rhs_all], axis=1)  # for rotation

    feat = np.concatenate([x64, sq[:, None], np.ones((N, 1))], axis=1)
    feat = feat.astype(np.float32).reshape(CORES, NCHUNK, PT, FDIM)
    feat = np.ascontiguousarray(feat.transpose(0, 2, 1, 3))
    ohf = oh.astype(np.float32).reshape(CORES, NCHUNK, PT, NCLS)
    ohf = np.ascontiguousarray(ohf.transpose(0, 2, 1, 3))

    in_maps = []
    for cc in range(CORES):
        in_maps.append({
            "lhst": np.ascontiguousarray(
                lhst_all[:, cc * ROWS:(cc + 1) * ROWS]),
            "rhs": np.ascontiguousarray(
                rhs2[:, cc * ROWS:cc * ROWS + RHSW]),
            "clsoh": ohf[cc],
            "clsft": feat[cc],
        })
    return in_maps


def _exact_q(x: np.ndarray, label: np.ndarray) -> float:
    """Exact Q = sum over ordered diff-label pairs of relu(1-d)^2 (fp64,
    chunked).  Only runs when the device flags a potential margin pair."""
    x64 = x.astype(np.float64)
    sq = (x64 * x64).sum(axis=1)
    q = 0.0
    step = 1024
    for a in range(0, N, step):
        d2 = sq[a:a + step, None] + sq[None, :] - 2.0 * (x64[a:a + step] @ x64.T)
        d = np.sqrt(np.maximum(d2, 0.0))
        diff = label[a:a + step, None] != label[None, :]
        r = np.maximum(MARGIN - d, 0.0)
        offdiag = np.arange(a, a + step)[:, None] != np.arange(N)[None, :]
        q += float((r * r)[diff & offdiag].sum())
    return q


def finish(results, x: np.ndarray, label: np.ndarray) -> np.float32:
    cls = np.zeros((NCLS, FDIM), np.float64)
    for rr in results:
        cls += rr["cls"].astype(np.float64)
    M = cls[:, :D]
    S = cls[:, D]
    ncnt = cls[:, D + 1]
    P = float((2.0 * ncnt * S - 2.0 * (M * M).sum(axis=1)).sum())

    flagged = False
    for rr in results:
        v = rr["viol"]
        if (v[:, 0:RT * NTPR:2] > 0.0).any():          # ACT sums
            flagged = True
        if (v[:, 1:RT * NTPR:2] > -THETA).any():       # DVE maxes
            flagged = True
        if (v[:, RT * NTPR] > -THETA).any():           # extras max
            flagged = True
    Q = _exact_q(x, label) if flagged else 0.0

    return np.float32((P + Q) / (N * (N - 1)))


_NC_CACHE: dict = {}


def kernel(output: np.ndarray, label: np.ndarray) -> np.ndarray:
    x = np.asarray(output, dtype=np.float32)
    lab = np.asarray(label).astype(np.int64)
    assert x.shape == (N, D) and lab.shape == (N,)

    if "nc" not in _NC_CACHE:
        _NC_CACHE["nc"] = build_nc()
    nc = _NC_CACHE["nc"]

    in_maps = prep_inputs(x, lab)
    res = run_bass_kernel_spmd(nc, in_maps, core_ids=list(range(CORES)))
    loss = finish(res.results, x, lab)
    return np.asarray(loss, dtype=np.float32)

